# revision 29
# baseline (speedup 1.0000x reference)
"""MoE layer (8 experts, top-2 sigmoid routing, SwiGLU experts + shared expert)
on 8 TRN2 NeuronCores.

Strategy (expert-parallel, host-side token dispatch):
  - Router (sigmoid(x @ gate_w.T), top-2, weight normalization) is tiny
    (~50 MFLOP) and runs on the host; it determines the all-to-all dispatch.
  - Core c owns expert c: it gets the tokens routed to expert c (gathered and
    zero-padded to a common capacity m_pad) plus expert c's Wi/Wo.
  - The shared expert is data-parallel: core c also processes tokens
    [c*512, (c+1)*512) with the (replicated) shared weights.
  - Device kernel computes the two SwiGLU MLP passes in bf16 (fp32 PSUM
    accumulation), feature-major layout (features on partitions, tokens on the
    free dim) so no on-device transposes are needed.
  - Host combine: out[t] = shared_out[t] + sum_e cw[e,t] * expert_out[e][t]
    (the combine weights are applied on the host during the scatter-add).

Trace-driven layout decisions (see per-line comments):
  - Every input tensor is host-pre-tiled into its exact SBUF layout so each
    one is a SINGLE DMA_DIRECT2D: issue cost on the sync queue is ~0.62us per
    DMA, and the old per-k-tile scheme (60 issues) delayed the first real
    matmul to ~15.7us.  Packed: the shared job's data (xs + first swi pair)
    is in SBUF by ~10us.
  - All input DMAs ride ONE in-order HWDGE queue (nc.sync), ordered exactly
    by first use: two queues would round-robin SDMA packets and halve the
    effective bandwidth of the critical early stream.
  - Outputs are bf16 (host upcasts): halves output bytes and the tail drain.
  - Warm-up is 9 matmuls (~3.6us cold): just enough to cover the HAM
    clock-gate SHORT window (~3.4us) so real matmuls run at 2.4 GHz, without
    delaying them (the old 16-MM warm-up overshot the data arrival by ~5us).
  - Chunk pipeline emits Wi(c+1) BEFORE Wo(c) for every c (including c=0):
    the PE always has independent matmul work while ACT/DVE finish chunk c's
    SwiGLU, so there is no bubble at any chunk boundary.
"""

from contextlib import ExitStack

import ml_dtypes
import numpy as np

import concourse.tile as tile
from concourse import bacc, mybir
from concourse.bass_utils import run_bass_kernel_spmd

E, TOPK, H, I = 8, 2, 768, 1152
I2 = 2 * I
T = 4096
N_CORES = 8
TS = T // N_CORES  # shared-expert tokens per core
P = 128
KH = H // P    # 6 contraction tiles over H
KI = I // P    # 9 contraction tiles over I
BF16 = mybir.dt.bfloat16
F32 = mybir.dt.float32
MAXN = 512     # max tokens per matmul chunk (one fp32 PSUM bank)
TSH0 = 256     # shared job is 2 chunks of 256: the first chunk then only
               # needs 0.79 MB (xs half + first swi pair) -> real matmuls
               # start ~13.0us (sync doorbell ~8.7us + transfer + ~1.9us
               # completion/semaphore latency).  (Tried {233,279} + 11 warm
               # matmuls to start ~0.3us earlier: measured 1.1us WORSE.)
TSH1 = TS - TSH0
WARM_MM = 10   # cold N=512 matmuls ~4.3us: covers the 3.4us HAM window and
               # ends ~when the first chunk's data lands (~12us with the
               # scalar-ring early loads)

_BUILD_CACHE: dict = {}
LAST_RESULTS = None  # BassKernelResults of the most recent device run
USE_SILU = True  # native ACT Silu on HW; set False for CoreSim (not implemented there)


def _ensure_axon_ntff_hook():
    """This image's `antenv` lacks the `axon_hooks` module that
    run_bass_kernel_spmd imports when NTFF tracing is requested (BASS_TRACE=1).
    Install an equivalent shim so profiling works instead of crashing."""
    try:
        import antenv.axon_hooks  # noqa: F401
        return
    except ImportError:
        pass
    import sys
    import types
    try:
        import antenv
    except ImportError:
        return
    mod = types.ModuleType("antenv.axon_hooks")
    holder = {"hook": None}
    mod.set_axon_ntff_profile_hook = lambda h: holder.__setitem__("hook", h)
    mod.get_axon_ntff_profile_hook = lambda: holder["hook"]
    sys.modules["antenv.axon_hooks"] = mod
    antenv.axon_hooks = mod
    so_path = "/opt/axon/libaxon_pjrt.so"
    try:
        import os
        if os.path.exists(so_path):
            from trn_agent_boot.trn_boot import _ntff_profile_via_ctypes
            hook = _ntff_profile_via_ctypes(so_path)
            if hook is not None:
                mod.set_axon_ntff_profile_hook(hook)
    except Exception:
        pass  # hook stays None; bass_utils logs a warning and skips tracing


def _chunk_sizes(m: int) -> list[int]:
    """Split m into chunks of <=512 tokens, all >=233 (below that a chunk is
    LDWEIGHTS-bound: LDW ~97ns vs N/2.4ns streaming).  Total stream time is
    size-independent while every chunk stays matmul-bound, so make the LAST
    chunk the minimum (233): the final Wo drain (copy+DMA after the last
    matmul) scales with the last chunk's width."""
    if 978 <= m <= 1257:
        # stream time is size-independent while chunks stay matmul-bound
        # (>=233); a narrow LAST chunk shrinks the final copy+DMA drain
        return [MAXN, m - MAXN - 233, 233]
    n = -(-m // MAXN)
    base, rem = divmod(m, n)
    return [base + 1] * rem + [base] * (n - rem)


def _build(m_pad: int):
    nc = bacc.Bacc("TRN2", target_bir_lowering=False, debug=False,
                   num_devices=N_CORES)

    # All inputs arrive host-pre-tiled in their exact SBUF layout -> each is
    # one contiguous-per-partition DMA.
    xs0 = nc.dram_tensor("xs0", [P, KH * TSH0], BF16, kind="ExternalInput").ap()
    xs1 = nc.dram_tensor("xs1", [P, KH * TSH1], BF16, kind="ExternalInput").ap()
    # swi[ft] = the ft-th (A,B) pair: A cols [0:H], B cols [H:2H]; pairs are
    # separate DMAs so the k-chains' data lands in consumption order.
    swi = nc.dram_tensor("swi", [KI, P, 2 * H], BF16, kind="ExternalInput").ap()
    xe = nc.dram_tensor("xe", [P, KH * m_pad], BF16, kind="ExternalInput").ap()
    # expert Wi as (A,B)-interleaved f-tile pairs, same layout as swi: the
    # Wi chains consume pairs in exactly the order they stream in
    wie = nc.dram_tensor("wie", [KI, P, 2 * H], BF16, kind="ExternalInput").ap()
    swo = nc.dram_tensor("swo", [P, KI * H], BF16, kind="ExternalInput").ap()
    wo = nc.dram_tensor("wo", [P, KI * H], BF16, kind="ExternalInput").ap()
    ye = nc.dram_tensor("ye", [H, m_pad], BF16, kind="ExternalOutput").ap()
    ys = nc.dram_tensor("ys", [H, TS], BF16, kind="ExternalOutput").ap()

    with ExitStack() as ctx:
        tc = ctx.enter_context(tile.TileContext(nc))
        wpool = ctx.enter_context(tc.tile_pool(name="weights", bufs=1))
        apool = ctx.enter_context(tc.tile_pool(name="act", bufs=3))
        spool = ctx.enter_context(tc.tile_pool(name="silu", bufs=4))
        ypool = ctx.enter_context(tc.tile_pool(name="y", bufs=3))
        # all 8 PSUM banks in one pool; the warm-up tiles share the "ps" tag
        # so their 2 banks recycle into the working set after the lead-in
        psum = ctx.enter_context(tc.tile_pool(name="psum", bufs=8, space="PSUM"))

        # ---- input loads: one in-order HWDGE queue, first-use order.
        # (Tried: first two loads via the GpSimd SWDGE queue to beat the
        # sync engine's later preamble — measured WORSE: SWDGE's first byte
        # moved ~1.2us later than sync's, and the two queues split SDMA
        # bandwidth, delaying the critical first-chunk data by ~3us.) ----
        def load(name, src, cols, eng=None):
            t = wpool.tile([P, cols], BF16, tag=name, name=name)
            (eng or nc.sync).dma_start(t[:], src)
            return t

        # the two tiles gating the first chunk ride the SCALAR HWDGE ring:
        # the scalar engine exits the runtime preamble ~1.1us before sync,
        # so their doorbell (and completion) land earlier; the sync stream
        # only competes for SDMA bandwidth after ~8.7us
        xs0_t = load("xs0", xs0, KH * TSH0, eng=nc.scalar)
        swi_t = [None] * KI
        swi_t[0] = load("swi0", swi[0], 2 * H, eng=nc.scalar)
        swi_t[1] = load("swi1", swi[1], 2 * H)
        swi_t[2] = load("swi2", swi[2], 2 * H)
        xs1_t = load("xs1", xs1, KH * TSH1)
        for ft in range(3, KI):
            swi_t[ft] = load(f"swi{ft}", swi[ft], 2 * H)
        # swo BEFORE the expert stream: with the 256-token shared chunks,
        # Wo(0) runs at ~37us and a late swo (measured) stalls the PE 4.3us
        # AND re-throttles the HAM clock gate (another ~2.5us at half rate)
        swo_t = load("swo", swo, KI * H)
        xe_t = load("xe", xe, KH * m_pad)
        wie_t = [load(f"wie{ft}", wie[ft], 2 * H) for ft in range(KI)]
        wo_t = load("wo", wo, KI * H)

        # ---- HAM warm-up: scratch matmuls fill the DMA lead-in so the
        # clock gate un-throttles (4/8 -> 8/8) before the real matmuls ----
        warm_sink = nc.dram_tensor("warm_sink", [P, MAXN], BF16).ap()
        warm_sb = wpool.tile([P, MAXN], BF16, tag="warm", name="warm")
        # memset on GpSimd: it exits the runtime preamble first (~6.0us vs
        # ~6.4us for DVE), so the first warm-up matmul issues sooner
        nc.gpsimd.memset(warm_sb[:], 0.0)
        wps = [psum.tile([P, MAXN], F32, tag="ps", name=f"wps{i}")
               for i in range(2)]
        for i in range(WARM_MM):
            nc.tensor.matmul(wps[i % 2], lhsT=warm_sb[:, :P], rhs=warm_sb[:],
                             start=True, stop=True)
        warm_out = ypool.tile([P, MAXN], BF16, tag="y", name="warm_out")
        nc.scalar.copy(warm_out[:], wps[(WARM_MM - 1) % 2])
        nc.gpsimd.dma_start(warm_sink[:], warm_out[:])

        # accessors: x(kt, x_off, sz) -> rhs AP; wa/wb/wo(ft|ht, kt) -> lhsT AP
        def sh_acc(xt, w):
            return dict(
                x=lambda kt, off, sz: xt[:, kt * w + off: kt * w + off + sz],
                wa=lambda ft, kt: swi_t[ft][:, kt * P:(kt + 1) * P],
                wb=lambda ft, kt: swi_t[ft][:, H + kt * P: H + (kt + 1) * P],
                wo=lambda ht, kt: swo_t[:, kt * H + ht * P: kt * H + (ht + 1) * P],
            )
        ex = dict(
            x=lambda kt, off, sz: xe_t[:, kt * m_pad + off: kt * m_pad + off + sz],
            wa=lambda ft, kt: wie_t[ft][:, kt * P:(kt + 1) * P],
            wb=lambda ft, kt: wie_t[ft][:, H + kt * P: H + (kt + 1) * P],
            wo=lambda ht, kt: wo_t[:, kt * H + ht * P: kt * H + (ht + 1) * P],
        )

        # (accessors, y_dram, out_off, x_off, chunk_sz, silu_on_first)
        chunks = [
            (sh_acc(xs0_t, TSH0), ys, 0, 0, TSH0, True),
            (sh_acc(xs1_t, TSH1), ys, TSH0, 0, TSH1, True),
        ]
        off = 0
        for sz in _chunk_sizes(m_pad):
            chunks.append((ex, ye, off, off, sz, False))
            off += sz

        def emit_wi(c):
            acc, yd, out_off, x_off, sz, sfirst = chunks[c]
            act = apool.tile([P, KI, MAXN], BF16, tag="act", name="act")[:, :, :sz]
            for ft in range(KI):
                ps_a = psum.tile([P, MAXN], F32, tag="ps", name="ps_a")[:, :sz]
                for kt in range(KH):
                    nc.tensor.matmul(ps_a, lhsT=acc["wa"](ft, kt),
                                     rhs=acc["x"](kt, x_off, sz),
                                     start=(kt == 0), stop=(kt == KH - 1))
                ps_b = psum.tile([P, MAXN], F32, tag="ps", name="ps_b")[:, :sz]
                for kt in range(KH):
                    nc.tensor.matmul(ps_b, lhsT=acc["wb"](ft, kt),
                                     rhs=acc["x"](kt, x_off, sz),
                                     start=(kt == 0), stop=(kt == KH - 1))
                sl = spool.tile([P, MAXN], F32, tag="silu", name="sl")[:, :sz]
                ps_s, ps_m = (ps_a, ps_b) if sfirst else (ps_b, ps_a)
                if USE_SILU:
                    # act = silu(s) * m: one ACT op + one DVE mul; PSUM banks
                    # are freed one op earlier than the sigmoid+2-mul form
                    nc.scalar.activation(sl, ps_s,
                                         mybir.ActivationFunctionType.Silu)
                    nc.vector.tensor_mul(act[:, ft, :], sl, ps_m)
                else:
                    # CoreSim fallback: silu(s) = s * sigmoid(s)
                    tmp = spool.tile([P, MAXN], F32, tag="silu2",
                                     name="tmp")[:, :sz]
                    nc.scalar.activation(sl, ps_s,
                                         mybir.ActivationFunctionType.Sigmoid)
                    nc.vector.tensor_mul(tmp, sl, ps_s)
                    nc.vector.tensor_mul(act[:, ft, :], tmp, ps_m)
            return act

        def emit_wo(c, act, last=False):
            acc, yd, out_off, x_off, sz, sfirst = chunks[c]
            off = out_off
            for ht in range(KH):
                ps_y = psum.tile([P, MAXN], F32, tag="ps", name="ps_y")[:, :sz]
                for kt in range(KI):
                    nc.tensor.matmul(ps_y, lhsT=acc["wo"](ht, kt),
                                     rhs=act[:, kt, :],
                                     start=(kt == 0), stop=(kt == KI - 1))
                yt = ypool.tile([P, MAXN], BF16, tag="y", name="yt")[:, :sz]
                # bf16 output (host upcasts): halves output DMA bytes.
                # Copy on the (otherwise idle) Scalar engine so DVE mul
                # throughput isn't what frees PSUM banks; output DMA on the
                # GpSimd SWDGE queue to stay off the input HWDGE stream.
                # Exception: the LAST chunk's outputs ride the (long-idle)
                # HWDGE queue so the tail isn't serialized behind the SWDGE
                # queue's earlier packets.
                nc.scalar.copy(yt, ps_y)
                dma_eng = nc.sync if last else nc.gpsimd
                dma_eng.dma_start(
                    yd.rearrange("(o p) m -> p o m", p=P)[:, ht, off:off + sz], yt)

        # software pipeline: Wi(c+1) is emitted before Wo(c) for every c, so
        # the PE always has independent matmul work while ACT/DVE finish
        # chunk c's SwiGLU.  (All weights are resident long before Wo(0)
        # runs, so there is no head-of-line risk from the expert stream.)
        n = len(chunks)
        acts = [None] * n
        acts[0] = emit_wi(0)
        for c in range(1, n):
            acts[c] = emit_wi(c)
            emit_wo(c - 1, acts[c - 1])
        emit_wo(n - 1, acts[n - 1], last=True)

    nc.compile()
    return nc


def _feat_major(a: np.ndarray, ktiles: int) -> np.ndarray:
    """(ktiles*P, W) -> (P, ktiles*W): k-tiles side by side, features on
    partitions — the exact SBUF layout, so the load is one DMA."""
    kp, w = a.shape
    assert kp == ktiles * P
    return np.ascontiguousarray(
        a.reshape(ktiles, P, w).transpose(1, 0, 2).reshape(P, ktiles * w))


def _pack_swi(swiT: np.ndarray) -> np.ndarray:
    """(H, 2I) -> (KI, P, 2H): per-f-tile (A,B) pairs, each pair contiguous."""
    proj = swiT[:, :I].reshape(KH, P, KI, P)
    gate = swiT[:, I:].reshape(KH, P, KI, P)
    A = proj.transpose(2, 1, 0, 3).reshape(KI, P, H)
    B = gate.transpose(2, 1, 0, 3).reshape(KI, P, H)
    return np.ascontiguousarray(np.concatenate([A, B], axis=2))


def _route(x, gate_w, correction_bias):
    logits = 1.0 / (1.0 + np.exp(-(x @ gate_w.T), dtype=np.float32))  # (T, E)
    sel = logits + correction_bias[None, :]
    order = np.argsort(-sel, axis=1, kind="stable")[:, :TOPK]  # ties -> low index
    w = np.take_along_axis(logits, order, axis=1)
    w = (w / w.sum(axis=1, keepdims=True)).astype(np.float32)
    return order, w


def kernel(**inputs) -> np.ndarray:
    x = np.asarray(inputs["x"], np.float32)
    gate_w = np.asarray(inputs["gate_w"], np.float32)
    bias = np.asarray(inputs["correction_bias"], np.float32)
    Wi = np.asarray(inputs["Wi"], np.float32)
    Wo = np.asarray(inputs["Wo"], np.float32)
    shared_Wi = np.asarray(inputs["shared_Wi"], np.float32)
    shared_Wo = np.asarray(inputs["shared_Wo"], np.float32)

    order, w = _route(x, gate_w, bias)

    idx_per_e, cw_per_e = [], []
    for e in range(E):
        mask = order == e  # (T, K)
        tok = mask.any(axis=1)
        rows = np.nonzero(tok)[0]
        kpos = np.argmax(mask[rows], axis=1)
        idx_per_e.append(rows)
        cw_per_e.append(w[rows, kpos].astype(np.float32))

    mx = max(len(r) for r in idx_per_e)
    m_pad = max(64, mx + (mx & 1))  # exact capacity, kept even for alignment

    bf = ml_dtypes.bfloat16
    xT = np.ascontiguousarray(x.T)  # (H, T) f32
    swi_p = _pack_swi(shared_Wi.T).astype(bf)                 # (KI, P, 2H)
    swo_p = _feat_major(shared_Wo.T, KI).astype(bf)           # (P, KI*H)

    in_maps = []
    for c in range(N_CORES):
        rows = idx_per_e[c]
        xe = np.zeros((H, m_pad), np.float32)
        xe[:, :len(rows)] = xT[:, rows]
        in_maps.append({
            "xs0": _feat_major(
                xT[:, c * TS: c * TS + TSH0], KH).astype(bf),
            "xs1": _feat_major(
                xT[:, c * TS + TSH0: (c + 1) * TS], KH).astype(bf),
            "swi": swi_p,
            "xe": _feat_major(xe, KH).astype(bf),
            "wie": _pack_swi(Wi[c]).astype(bf),
            "swo": swo_p,
            "wo": _feat_major(Wo[c], KI).astype(bf),
        })

    if m_pad not in _BUILD_CACHE:
        _BUILD_CACHE[m_pad] = _build(m_pad)
    nc = _BUILD_CACHE[m_pad]

    _ensure_axon_ntff_hook()
    res = run_bass_kernel_spmd(nc, in_maps, list(range(N_CORES)))
    global LAST_RESULTS
    LAST_RESULTS = res

    out = np.zeros((T, H), np.float32)
    for c in range(N_CORES):
        r = res.results[c]
        out[c * TS:(c + 1) * TS] += r["ys"].astype(np.float32).T
        rows = idx_per_e[c]
        if len(rows):
            out[rows] += (r["ye"][:, :len(rows)].astype(np.float32).T
                          * cw_per_e[c][:, None])
    return out


# revision 31
# speedup vs baseline: 1.2257x; 1.2257x over previous
"""MoE layer (8 experts, top-2 sigmoid routing, SwiGLU experts + shared expert)
on 8 TRN2 NeuronCores.

Strategy (expert-parallel, host-side token dispatch):
  - Router (sigmoid(x @ gate_w.T), top-2, weight normalization) is tiny
    (~50 MFLOP) and runs on the host; it determines the all-to-all dispatch.
  - Core c owns expert c: it gets the tokens routed to expert c (gathered and
    zero-padded to a common capacity m_pad) plus expert c's Wi/Wo.
  - The shared expert is data-parallel: core c also processes tokens
    [c*512, (c+1)*512) with the (replicated) shared weights.
  - Device kernel computes the two SwiGLU MLP passes in bf16 (fp32 PSUM
    accumulation), feature-major layout (features on partitions, tokens on the
    free dim) so no on-device transposes are needed.
  - Host combine: out[t] = shared_out[t] + sum_e cw[e,t] * expert_out[e][t]
    (the combine weights are applied on the host during the scatter-add).

Trace-driven layout decisions (see per-line comments):
  - Every input tensor is host-pre-tiled into its exact SBUF layout so each
    one is a SINGLE DMA_DIRECT2D: issue cost on the sync queue is ~0.62us per
    DMA, and the old per-k-tile scheme (60 issues) delayed the first real
    matmul to ~15.7us.  Packed: the shared job's data (xs + first swi pair)
    is in SBUF by ~10us.
  - All input DMAs ride ONE in-order HWDGE queue (nc.sync), ordered exactly
    by first use: two queues would round-robin SDMA packets and halve the
    effective bandwidth of the critical early stream.
  - Outputs are bf16 (host upcasts): halves output bytes and the tail drain.
  - Warm-up is 9 matmuls (~3.6us cold): just enough to cover the HAM
    clock-gate SHORT window (~3.4us) so real matmuls run at 2.4 GHz, without
    delaying them (the old 16-MM warm-up overshot the data arrival by ~5us).
  - Chunk pipeline emits Wi(c+1) BEFORE Wo(c) for every c (including c=0):
    the PE always has independent matmul work while ACT/DVE finish chunk c's
    SwiGLU, so there is no bubble at any chunk boundary.
"""

from contextlib import ExitStack

import ml_dtypes
import numpy as np

import concourse.tile as tile
from concourse import bacc, mybir
from concourse.bass_utils import run_bass_kernel_spmd

E, TOPK, H, I = 8, 2, 768, 1152
I2 = 2 * I
T = 4096
N_CORES = 8
TS = T // N_CORES  # shared-expert tokens per core
P = 128
KH = H // P    # 6 contraction tiles over H
KI = I // P    # 9 contraction tiles over I
BF16 = mybir.dt.bfloat16
F32 = mybir.dt.float32
MAXN = 512     # max tokens per matmul chunk (one fp32 PSUM bank)
TSH0 = 256     # shared job is 2 chunks of 256: the first chunk then only
               # needs 0.79 MB (xs half + first swi pair) -> real matmuls
               # start ~13.0us (sync doorbell ~8.7us + transfer + ~1.9us
               # completion/semaphore latency).  (Tried {233,279} + 11 warm
               # matmuls to start ~0.3us earlier: measured 1.1us WORSE.)
TSH1 = TS - TSH0
WARM_MM = 12   # cold N=512 matmuls ~5.1us: covers the 3.4us HAM window and
               # ends ~when the first chunk's data lands (measured ~13.0us)

_BUILD_CACHE: dict = {}
LAST_RESULTS = None  # BassKernelResults of the most recent device run
USE_SILU = True  # native ACT Silu on HW; set False for CoreSim (not implemented there)


def _ensure_axon_ntff_hook():
    """This image's `antenv` lacks the `axon_hooks` module that
    run_bass_kernel_spmd imports when NTFF tracing is requested (BASS_TRACE=1).
    Install an equivalent shim so profiling works instead of crashing."""
    try:
        import antenv.axon_hooks  # noqa: F401
        return
    except ImportError:
        pass
    import sys
    import types
    try:
        import antenv
    except ImportError:
        return
    mod = types.ModuleType("antenv.axon_hooks")
    holder = {"hook": None}
    mod.set_axon_ntff_profile_hook = lambda h: holder.__setitem__("hook", h)
    mod.get_axon_ntff_profile_hook = lambda: holder["hook"]
    sys.modules["antenv.axon_hooks"] = mod
    antenv.axon_hooks = mod
    so_path = "/opt/axon/libaxon_pjrt.so"
    try:
        import os
        if os.path.exists(so_path):
            from trn_agent_boot.trn_boot import _ntff_profile_via_ctypes
            hook = _ntff_profile_via_ctypes(so_path)
            if hook is not None:
                mod.set_axon_ntff_profile_hook(hook)
    except Exception:
        pass  # hook stays None; bass_utils logs a warning and skips tracing


def _chunk_sizes(m: int) -> list[int]:
    """Split m into chunks of <=512 tokens, all >=233 (below that a chunk is
    LDWEIGHTS-bound: LDW ~97ns vs N/2.4ns streaming).  Total stream time is
    size-independent while every chunk stays matmul-bound, so make the LAST
    chunk the minimum (233): the final Wo drain (copy+DMA after the last
    matmul) scales with the last chunk's width."""
    if 978 <= m <= 1257:
        # stream time is size-independent while chunks stay matmul-bound
        # (>=233); a narrow LAST chunk shrinks the final copy+DMA drain
        return [MAXN, m - MAXN - 233, 233]
    n = -(-m // MAXN)
    base, rem = divmod(m, n)
    return [base + 1] * rem + [base] * (n - rem)


def _build(m_pad: int):
    nc = bacc.Bacc("TRN2", target_bir_lowering=False, debug=False,
                   num_devices=N_CORES)

    # All inputs arrive host-pre-tiled in their exact SBUF layout -> each is
    # one contiguous-per-partition DMA.
    xs0 = nc.dram_tensor("xs0", [P, KH * TSH0], BF16, kind="ExternalInput").ap()
    xs1 = nc.dram_tensor("xs1", [P, KH * TSH1], BF16, kind="ExternalInput").ap()
    # swi[ft] = the ft-th (A,B) pair: A cols [0:H], B cols [H:2H]; pairs are
    # separate DMAs so the k-chains' data lands in consumption order.
    swi = nc.dram_tensor("swi", [KI, P, 2 * H], BF16, kind="ExternalInput").ap()
    xe = nc.dram_tensor("xe", [P, KH * m_pad], BF16, kind="ExternalInput").ap()
    # expert Wi as (A,B)-interleaved f-tile pairs, same layout as swi: the
    # Wi chains consume pairs in exactly the order they stream in
    wie = nc.dram_tensor("wie", [KI, P, 2 * H], BF16, kind="ExternalInput").ap()
    swo = nc.dram_tensor("swo", [P, KI * H], BF16, kind="ExternalInput").ap()
    wo = nc.dram_tensor("wo", [P, KI * H], BF16, kind="ExternalInput").ap()
    ye = nc.dram_tensor("ye", [H, m_pad], BF16, kind="ExternalOutput").ap()
    ys = nc.dram_tensor("ys", [H, TS], BF16, kind="ExternalOutput").ap()

    with ExitStack() as ctx:
        tc = ctx.enter_context(tile.TileContext(nc))
        wpool = ctx.enter_context(tc.tile_pool(name="weights", bufs=1))
        apool = ctx.enter_context(tc.tile_pool(name="act", bufs=3))
        spool = ctx.enter_context(tc.tile_pool(name="silu", bufs=4))
        ypool = ctx.enter_context(tc.tile_pool(name="y", bufs=3))
        # all 8 PSUM banks in one pool; the warm-up tiles share the "ps" tag
        # so their 2 banks recycle into the working set after the lead-in
        psum = ctx.enter_context(tc.tile_pool(name="psum", bufs=8, space="PSUM"))

        # ---- input loads: one in-order HWDGE queue, first-use order.
        # (Tried: first two loads via the GpSimd SWDGE queue to beat the
        # sync engine's later preamble — measured WORSE: SWDGE's first byte
        # moved ~1.2us later than sync's, and the two queues split SDMA
        # bandwidth, delaying the critical first-chunk data by ~3us.) ----
        def load(name, src, cols, eng=None):
            t = wpool.tile([P, cols], BF16, tag=name, name=name)
            (eng or nc.sync).dma_start(t[:], src)
            return t

        # (Tried routing these two via the scalar HWDGE ring to beat the
        # sync engine's later preamble: measured +29us — the scalar ring's
        # issue serializes disastrously with the ACT work.  Tried GpSimd
        # SWDGE too: first byte lands later and splits bandwidth.  The
        # single sync ring in first-use order is the optimum found.)
        xs0_t = load("xs0", xs0, KH * TSH0)
        swi_t = [None] * KI
        swi_t[0] = load("swi0", swi[0], 2 * H)
        swi_t[1] = load("swi1", swi[1], 2 * H)
        swi_t[2] = load("swi2", swi[2], 2 * H)
        xs1_t = load("xs1", xs1, KH * TSH1)
        for ft in range(3, KI):
            swi_t[ft] = load(f"swi{ft}", swi[ft], 2 * H)
        # swo BEFORE the expert stream: with the 256-token shared chunks,
        # Wo(0) runs at ~37us and a late swo (measured) stalls the PE 4.3us
        # AND re-throttles the HAM clock gate (another ~2.5us at half rate)
        swo_t = load("swo", swo, KI * H)
        xe_t = load("xe", xe, KH * m_pad)
        wie_t = [load(f"wie{ft}", wie[ft], 2 * H) for ft in range(KI)]
        wo_t = load("wo", wo, KI * H)

        # ---- HAM warm-up: scratch matmuls fill the DMA lead-in so the
        # clock gate un-throttles (4/8 -> 8/8) before the real matmuls ----
        warm_sink = nc.dram_tensor("warm_sink", [P, MAXN], BF16).ap()
        warm_sb = wpool.tile([P, MAXN], BF16, tag="warm", name="warm")
        # memset on GpSimd: it exits the runtime preamble first (~6.0us vs
        # ~6.4us for DVE), so the first warm-up matmul issues sooner
        nc.gpsimd.memset(warm_sb[:], 0.0)
        wps = [psum.tile([P, MAXN], F32, tag="ps", name=f"wps{i}")
               for i in range(2)]
        for i in range(WARM_MM):
            nc.tensor.matmul(wps[i % 2], lhsT=warm_sb[:, :P], rhs=warm_sb[:],
                             start=True, stop=True)
        warm_out = ypool.tile([P, MAXN], BF16, tag="y", name="warm_out")
        nc.scalar.copy(warm_out[:], wps[(WARM_MM - 1) % 2])
        nc.gpsimd.dma_start(warm_sink[:], warm_out[:])

        # accessors: x(kt, x_off, sz) -> rhs AP; wa/wb/wo(ft|ht, kt) -> lhsT AP
        def sh_acc(xt, w):
            return dict(
                x=lambda kt, off, sz: xt[:, kt * w + off: kt * w + off + sz],
                wa=lambda ft, kt: swi_t[ft][:, kt * P:(kt + 1) * P],
                wb=lambda ft, kt: swi_t[ft][:, H + kt * P: H + (kt + 1) * P],
                wo=lambda ht, kt: swo_t[:, kt * H + ht * P: kt * H + (ht + 1) * P],
            )
        ex = dict(
            x=lambda kt, off, sz: xe_t[:, kt * m_pad + off: kt * m_pad + off + sz],
            wa=lambda ft, kt: wie_t[ft][:, kt * P:(kt + 1) * P],
            wb=lambda ft, kt: wie_t[ft][:, H + kt * P: H + (kt + 1) * P],
            wo=lambda ht, kt: wo_t[:, kt * H + ht * P: kt * H + (ht + 1) * P],
        )

        # (accessors, y_dram, out_off, x_off, chunk_sz, silu_on_first)
        chunks = [
            (sh_acc(xs0_t, TSH0), ys, 0, 0, TSH0, True),
            (sh_acc(xs1_t, TSH1), ys, TSH0, 0, TSH1, True),
        ]
        off = 0
        for sz in _chunk_sizes(m_pad):
            chunks.append((ex, ye, off, off, sz, False))
            off += sz

        def emit_wi(c):
            acc, yd, out_off, x_off, sz, sfirst = chunks[c]
            act = apool.tile([P, KI, MAXN], BF16, tag="act", name="act")[:, :, :sz]
            for ft in range(KI):
                ps_a = psum.tile([P, MAXN], F32, tag="ps", name="ps_a")[:, :sz]
                for kt in range(KH):
                    nc.tensor.matmul(ps_a, lhsT=acc["wa"](ft, kt),
                                     rhs=acc["x"](kt, x_off, sz),
                                     start=(kt == 0), stop=(kt == KH - 1))
                ps_b = psum.tile([P, MAXN], F32, tag="ps", name="ps_b")[:, :sz]
                for kt in range(KH):
                    nc.tensor.matmul(ps_b, lhsT=acc["wb"](ft, kt),
                                     rhs=acc["x"](kt, x_off, sz),
                                     start=(kt == 0), stop=(kt == KH - 1))
                sl = spool.tile([P, MAXN], F32, tag="silu", name="sl")[:, :sz]
                ps_s, ps_m = (ps_a, ps_b) if sfirst else (ps_b, ps_a)
                if USE_SILU:
                    # act = silu(s) * m: one ACT op + one DVE mul; PSUM banks
                    # are freed one op earlier than the sigmoid+2-mul form
                    nc.scalar.activation(sl, ps_s,
                                         mybir.ActivationFunctionType.Silu)
                    nc.vector.tensor_mul(act[:, ft, :], sl, ps_m)
                else:
                    # CoreSim fallback: silu(s) = s * sigmoid(s)
                    tmp = spool.tile([P, MAXN], F32, tag="silu2",
                                     name="tmp")[:, :sz]
                    nc.scalar.activation(sl, ps_s,
                                         mybir.ActivationFunctionType.Sigmoid)
                    nc.vector.tensor_mul(tmp, sl, ps_s)
                    nc.vector.tensor_mul(act[:, ft, :], tmp, ps_m)
            return act

        def emit_wo(c, act, last=False):
            acc, yd, out_off, x_off, sz, sfirst = chunks[c]
            off = out_off
            for ht in range(KH):
                ps_y = psum.tile([P, MAXN], F32, tag="ps", name="ps_y")[:, :sz]
                for kt in range(KI):
                    nc.tensor.matmul(ps_y, lhsT=acc["wo"](ht, kt),
                                     rhs=act[:, kt, :],
                                     start=(kt == 0), stop=(kt == KI - 1))
                yt = ypool.tile([P, MAXN], BF16, tag="y", name="yt")[:, :sz]
                # bf16 output (host upcasts): halves output DMA bytes.
                # Copy on the (otherwise idle) Scalar engine so DVE mul
                # throughput isn't what frees PSUM banks; output DMA on the
                # GpSimd SWDGE queue to stay off the input HWDGE stream.
                # Exception: the LAST chunk's outputs ride the (long-idle)
                # HWDGE queue so the tail isn't serialized behind the SWDGE
                # queue's earlier packets.
                nc.scalar.copy(yt, ps_y)
                dma_eng = nc.sync if last else nc.gpsimd
                dma_eng.dma_start(
                    yd.rearrange("(o p) m -> p o m", p=P)[:, ht, off:off + sz], yt)

        # software pipeline: Wi(c+1) is emitted before Wo(c) for every c, so
        # the PE always has independent matmul work while ACT/DVE finish
        # chunk c's SwiGLU.  (All weights are resident long before Wo(0)
        # runs, so there is no head-of-line risk from the expert stream.)
        n = len(chunks)
        acts = [None] * n
        acts[0] = emit_wi(0)
        for c in range(1, n):
            acts[c] = emit_wi(c)
            emit_wo(c - 1, acts[c - 1])
        emit_wo(n - 1, acts[n - 1], last=True)

    nc.compile()
    return nc


def _feat_major(a: np.ndarray, ktiles: int) -> np.ndarray:
    """(ktiles*P, W) -> (P, ktiles*W): k-tiles side by side, features on
    partitions — the exact SBUF layout, so the load is one DMA."""
    kp, w = a.shape
    assert kp == ktiles * P
    return np.ascontiguousarray(
        a.reshape(ktiles, P, w).transpose(1, 0, 2).reshape(P, ktiles * w))


def _pack_swi(swiT: np.ndarray) -> np.ndarray:
    """(H, 2I) -> (KI, P, 2H): per-f-tile (A,B) pairs, each pair contiguous."""
    proj = swiT[:, :I].reshape(KH, P, KI, P)
    gate = swiT[:, I:].reshape(KH, P, KI, P)
    A = proj.transpose(2, 1, 0, 3).reshape(KI, P, H)
    B = gate.transpose(2, 1, 0, 3).reshape(KI, P, H)
    return np.ascontiguousarray(np.concatenate([A, B], axis=2))


def _route(x, gate_w, correction_bias):
    logits = 1.0 / (1.0 + np.exp(-(x @ gate_w.T), dtype=np.float32))  # (T, E)
    sel = logits + correction_bias[None, :]
    order = np.argsort(-sel, axis=1, kind="stable")[:, :TOPK]  # ties -> low index
    w = np.take_along_axis(logits, order, axis=1)
    w = (w / w.sum(axis=1, keepdims=True)).astype(np.float32)
    return order, w


def kernel(**inputs) -> np.ndarray:
    x = np.asarray(inputs["x"], np.float32)
    gate_w = np.asarray(inputs["gate_w"], np.float32)
    bias = np.asarray(inputs["correction_bias"], np.float32)
    Wi = np.asarray(inputs["Wi"], np.float32)
    Wo = np.asarray(inputs["Wo"], np.float32)
    shared_Wi = np.asarray(inputs["shared_Wi"], np.float32)
    shared_Wo = np.asarray(inputs["shared_Wo"], np.float32)

    order, w = _route(x, gate_w, bias)

    idx_per_e, cw_per_e = [], []
    for e in range(E):
        mask = order == e  # (T, K)
        tok = mask.any(axis=1)
        rows = np.nonzero(tok)[0]
        kpos = np.argmax(mask[rows], axis=1)
        idx_per_e.append(rows)
        cw_per_e.append(w[rows, kpos].astype(np.float32))

    mx = max(len(r) for r in idx_per_e)
    m_pad = max(64, mx + (mx & 1))  # exact capacity, kept even for alignment

    bf = ml_dtypes.bfloat16
    xT = np.ascontiguousarray(x.T)  # (H, T) f32
    swi_p = _pack_swi(shared_Wi.T).astype(bf)                 # (KI, P, 2H)
    swo_p = _feat_major(shared_Wo.T, KI).astype(bf)           # (P, KI*H)

    in_maps = []
    for c in range(N_CORES):
        rows = idx_per_e[c]
        xe = np.zeros((H, m_pad), np.float32)
        xe[:, :len(rows)] = xT[:, rows]
        in_maps.append({
            "xs0": _feat_major(
                xT[:, c * TS: c * TS + TSH0], KH).astype(bf),
            "xs1": _feat_major(
                xT[:, c * TS + TSH0: (c + 1) * TS], KH).astype(bf),
            "swi": swi_p,
            "xe": _feat_major(xe, KH).astype(bf),
            "wie": _pack_swi(Wi[c]).astype(bf),
            "swo": swo_p,
            "wo": _feat_major(Wo[c], KI).astype(bf),
        })

    if m_pad not in _BUILD_CACHE:
        _BUILD_CACHE[m_pad] = _build(m_pad)
    nc = _BUILD_CACHE[m_pad]

    _ensure_axon_ntff_hook()
    res = run_bass_kernel_spmd(nc, in_maps, list(range(N_CORES)))
    global LAST_RESULTS
    LAST_RESULTS = res

    out = np.zeros((T, H), np.float32)
    for c in range(N_CORES):
        r = res.results[c]
        out[c * TS:(c + 1) * TS] += r["ys"].astype(np.float32).T
        rows = idx_per_e[c]
        if len(rows):
            out[rows] += (r["ye"][:, :len(rows)].astype(np.float32).T
                          * cw_per_e[c][:, None])
    return out
